# revision 53
# baseline (speedup 1.0000x reference)
"""Trainium2 Bass kernel for nn_AFAA_31860067401960 — direct-band variant.

The depthwise 3x3 conv + Haar DWT are fused into four 4x4 stride-2
convolutions computed directly on PE: x is loaded twice with a one-row
shift on the two partition halves (K = 2 row-taps x 64 channels), so each
band-pair needs only 8 accumulating matmuls per output tile, with
M = 128 = (2 bands x 64 ch) — which is exactly the K-half layout the qkv
matmul wants. No intermediate d tensor exists at all.

The rest (Gram + per-batch AllReduce + masked softmax + fused
proj_w@(A+I)@v) matches the previous variant.
"""

import sys

sys.path.insert(0, "/opt/trn_rl_repo")

import numpy as np
import ml_dtypes

from concourse import bacc, bass, mybir, tile
from concourse.bass_utils import run_bass_kernel_spmd

F32 = mybir.dt.float32
BF16 = mybir.dt.bfloat16
AF = mybir.ActivationFunctionType
AX = mybir.AxisListType

B, C, H, W = 2, 64, 512, 512
HEADS, DH = 8, 8
CH = 128          # full-res rows per core
HL = 64           # low-res rows per core
WL = 256
NG = 16           # groups per core (4 low-res rows each)
BN_EPS = 1e-5
NORM_EPS = 1e-12

PHASES = [(r, w2) for r in range(2) for w2 in range(2)]
SGN = np.array(
    [[[1, 1], [1, 1]], [[1, 1], [-1, -1]], [[1, -1], [1, -1]], [[1, -1], [-1, 1]]],
    dtype=np.float32,
)


def _build_nc():
    nc = bacc.Bacc(None)

    # x param rows 0..129 = chunk full-res rows -1..128 (host-padded halo)
    xp = nc.declare_dram_parameter("x", [C, CH + 2, W], BF16, isOutput=False)
    bandw = nc.declare_dram_parameter("bandw", [128, 16 * 128], BF16, isOutput=False)
    qkvt1 = nc.declare_dram_parameter("qkvt1", [128, 192], BF16, isOutput=False)
    qkvt2 = nc.declare_dram_parameter("qkvt2", [128, 192], BF16, isOutput=False)
    vecs = nc.declare_dram_parameter("vecs", [128, 8], F32, isOutput=False)
    aux = nc.declare_dram_parameter("aux", [128, 256], F32, isOutput=False)
    i128 = nc.declare_dram_parameter("i128", [128, 128], F32, isOutput=False)
    identb = nc.declare_dram_parameter("identb", [128, 128], BF16, isOutput=False)

    yl_e = nc.declare_dram_parameter("yl", [C, HL, WL], BF16, isOutput=True)
    lh_e = nc.declare_dram_parameter("lh", [C, HL, WL], BF16, isOutput=True)
    hl_e = nc.declare_dram_parameter("hl", [C, HL, WL], BF16, isOutput=True)
    hh_e = nc.declare_dram_parameter("hh", [C, HL, WL], BF16, isOutput=True)
    out_e = nc.declare_dram_parameter("outp", [C, HL, WL], F32, isOutput=True)

    in_cc = nc.dram_tensor("in_cc", [128, 128], F32)
    out_cc = nc.dram_tensor("out_cc", [128, 128], F32)

    with tile.TileContext(nc) as tc:
        with (
            tc.tile_pool(name="consts", bufs=1) as cpool,
            tc.tile_pool(name="bpool", bufs=3) as bpool,
            tc.tile_pool(name="qkpool", bufs=3) as qkpool,
            tc.tile_pool(name="opool", bufs=8) as opool,
            tc.tile_pool(name="smalls", bufs=1) as spool,
        ):
            # persistent ping-pong x tiles: [128=(row-shift, c), 9 rows, 514]
            x_tiles = [
                cpool.tile([128, 9, W + 2], BF16, name=f"x_t{i}") for i in range(2)
            ]
            for xt in x_tiles:
                nc.vector.memset(xt[:, :, 0:1], 0.0)
                nc.vector.memset(xt[:, :, W + 1 : W + 2], 0.0)

            def load_x(g):
                x_t = x_tiles[g % 2]
                for dlt in range(2):
                    r0 = 8 * g + dlt
                    eng = nc.sync if dlt == 0 else nc.gpsimd
                    eng.dma_start(
                        x_t[dlt * 64 : (dlt + 1) * 64, :, 1 : W + 1],
                        xp[:, r0 : r0 + 9, :],
                    )
                return x_t

            load_x(0)
            bandw_sb = cpool.tile([128, 16 * 128], BF16)
            nc.sync.dma_start(bandw_sb[:], bandw[:])
            qkvt1_sb = cpool.tile([128, 192], BF16)
            nc.sync.dma_start(qkvt1_sb[:], qkvt1[:])
            qkvt2_sb = cpool.tile([128, 192], BF16)
            nc.sync.dma_start(qkvt2_sb[:], qkvt2[:])
            vecs_sb = cpool.tile([128, 8], F32)
            nc.sync.dma_start(vecs_sb[:], vecs[:])
            aux_sb = cpool.tile([128, 256], F32)
            nc.sync.dma_start(aux_sb[:], aux[:])
            i128_sb = cpool.tile([128, 128], F32)
            nc.sync.dma_start(i128_sb[:], i128[:])
            identb_sb = cpool.tile([128, 128], BF16)
            nc.sync.dma_start(identb_sb[:], identb[:])
            qks_sb = vecs_sb[:, 0:1]
            qkb_sb = vecs_sb[:, 1:2]
            bll_bias = vecs_sb[:, 2:3]   # [2*dw_b ; 0]
            vs_sb = vecs_sb[0:64, 3:4]
            vb_sb = vecs_sb[0:64, 4:5]
            temp_sb = vecs_sb[0:64, 5:6]
            projb_sb = vecs_sb[0:64, 6:7]
            projb2_sb = vecs_sb[:, 7:8]
            projT_sb = aux_sb[0:64, 0:64]
            mask_sb = aux_sb[0:64, 64:128]
            negbig_sb = aux_sb[0:64, 128:192]
            ones1_sb = aux_sb[0:1, 192:256]

            v_sb = cpool.tile([64, HL * WL], BF16)  # persistent v (bf16)

            mm_idx = 0
            n_gram = NG * 2 * 4

            with (
                tc.tile_pool(name="pb", bufs=2, space="PSUM") as pb,
                tc.tile_pool(name="pqk", bufs=2, space="PSUM") as pqk,
                tc.tile_pool(name="pv", bufs=2, space="PSUM") as pv,
                tc.tile_pool(name="pzt", bufs=1, space="PSUM") as pzt,
                tc.tile_pool(name="pg", bufs=1, space="PSUM") as pg,
            ):
                g_ps = pg.tile([128, 128], F32)  # Gram accumulator

                for g in range(NG):
                    x_t = x_tiles[g % 2] if g == 0 else load_x(g)

                    # ---- direct band computation: per band-pair, per
                    # half (2 low-res rows = 512 cols), 8 tap-pair matmuls
                    band_sbs = []
                    for bp in range(2):
                        b_sb = bpool.tile(
                            [128, 4, WL], BF16, tag=f"b{bp}", name=f"b{bp}_{g}"
                        )
                        for half in range(2):
                            w_ps = pb.tile([128, 2, WL], F32, tag="wps")
                            first = True
                            for tp in range(2):       # row-tap base 0 / 2
                                for tx in range(4):   # col tap
                                    blk = ((bp * 2 + tp) * 4 + tx) * 128
                                    rhs = x_t[
                                        :,
                                        2 * tp + 4 * half : 2 * tp + 4 * half + 3 : 2,
                                        tx : tx + 2 * WL - 1 : 2,
                                    ]
                                    nc.tensor.matmul(
                                        w_ps[:],
                                        bandw_sb[:, blk : blk + 128],
                                        rhs,
                                        start=first,
                                        stop=(tp == 1 and tx == 3),
                                    )
                                    first = False
                            dst = b_sb[:, half * 2 : half * 2 + 2, :]
                            if bp == 0:
                                nc.scalar.activation(
                                    dst, w_ps[:], AF.Identity, bias=bll_bias
                                )
                            else:
                                nc.vector.tensor_copy(dst, w_ps[:])
                        band_sbs.append(b_sb)

                    b01, b23 = band_sbs
                    lr0 = 4 * g
                    nc.sync.dma_start(yl_e[:, lr0 : lr0 + 4, :], b01[0:64, :, :])
                    nc.gpsimd.dma_start(lh_e[:, lr0 : lr0 + 4, :], b01[64:128, :, :])
                    nc.sync.dma_start(hl_e[:, lr0 : lr0 + 4, :], b23[0:64, :, :])
                    nc.gpsimd.dma_start(hh_e[:, lr0 : lr0 + 4, :], b23[64:128, :, :])

                    # ---- qkv + epilogue + gram, per 512-col chunk ----
                    for half in range(2):
                        rhs1 = b01[:, half * 2 : half * 2 + 2, :]
                        rhs2 = b23[:, half * 2 : half * 2 + 2, :]
                        qk_ps = pqk.tile([128, 512], F32, tag="qkps")
                        v_ps = pv.tile([64, 512], F32, tag="vps")
                        nc.tensor.matmul(
                            qk_ps[:], qkvt1_sb[:, 0:128], rhs1,
                            start=True, stop=False,
                        )
                        nc.tensor.matmul(
                            qk_ps[:], qkvt2_sb[:, 0:128], rhs2,
                            start=False, stop=True,
                        )
                        nc.tensor.matmul(
                            v_ps[:], qkvt1_sb[:, 128:192], rhs1,
                            start=True, stop=False,
                        )
                        nc.tensor.matmul(
                            v_ps[:], qkvt2_sb[:, 128:192], rhs2,
                            start=False, stop=True,
                        )
                        qk_sb = qkpool.tile([128, 512], BF16, tag="qk_sb")
                        nc.scalar.activation(
                            qk_sb[:], qk_ps[:], AF.Relu, bias=qkb_sb, scale=qks_sb
                        )
                        off = (4 * g + 2 * half) * WL
                        nc.scalar.activation(
                            v_sb[:, off : off + 512], v_ps[:], AF.Relu,
                            bias=vb_sb, scale=vs_sb,
                        )
                        zt_ps = pzt.tile([128, 512], BF16, tag="ztps")
                        for sub in range(4):
                            nc.tensor.transpose(
                                zt_ps[:, sub * 128 : (sub + 1) * 128],
                                qk_sb[:, sub * 128 : (sub + 1) * 128],
                                identb_sb[:],
                            )
                        zt_sb = qkpool.tile([128, 512], BF16, tag="zt_sb")
                        nc.vector.tensor_copy(zt_sb[:], zt_ps[:])
                        for sub in range(4):
                            blk2 = zt_sb[:, sub * 128 : (sub + 1) * 128]
                            nc.tensor.matmul(
                                g_ps[:], blk2, blk2,
                                start=(mm_idx == 0), stop=(mm_idx == n_gram - 1),
                            )
                            mm_idx += 1

                # ---- AllReduce of Gram over the 4 cores of this batch ----
                g_sb = spool.tile([128, 128], F32)
                nc.scalar.activation(g_sb[:], g_ps[:], AF.Copy)
                nc.sync.dma_start(in_cc[:], g_sb[:])
                nc.gpsimd.collective_compute(
                    "AllReduce",
                    mybir.AluOpType.add,
                    replica_groups=[[0, 1, 2, 3], [4, 5, 6, 7]],
                    ins=[in_cc[:]],
                    outs=[out_cc[:]],
                )
                gr_sb = spool.tile([128, 128], F32)
                nc.sync.dma_start(gr_sb[:], out_cc[:])

            with tc.tile_pool(name="pfin", bufs=6, space="PSUM") as pfin:
                # ---- norms / scaled logits / softmax ----
                dtmp = spool.tile([128, 128], F32)
                nc.vector.tensor_mul(dtmp[:], gr_sb[:], i128_sb[:])
                sq = spool.tile([128, 1], F32)
                nc.vector.reduce_sum(sq[:], dtmp[:], axis=AX.X)
                nrm = spool.tile([128, 1], F32)
                nc.scalar.sqrt(nrm[:], sq[:])
                nrm2 = spool.tile([128, 1], F32)
                nc.vector.tensor_scalar_max(nrm2[:], nrm[:], NORM_EPS)
                rinv = spool.tile([128, 1], F32)
                nc.vector.reciprocal(rinv[:], nrm2[:])
                rqt = spool.tile([64, 1], F32)
                nc.vector.tensor_mul(rqt[:], rinv[0:64, :], temp_sb)
                s_sb = spool.tile([64, 64], F32)
                nc.vector.tensor_scalar_mul(s_sb[:], gr_sb[0:64, 64:128], rqt[:, 0:1])
                rkT_ps = pfin.tile([1, 64], F32, tag="fin")
                nc.tensor.transpose(
                    rkT_ps[:], rinv[64:128, 0:1], i128_sb[64:128, 64:128]
                )
                rkT_sb = spool.tile([1, 64], F32)
                nc.scalar.activation(rkT_sb[:], rkT_ps[:], AF.Copy)
                rk_bc = pfin.tile([64, 64], F32, tag="fin")
                nc.tensor.matmul(rk_bc[:], ones1_sb, rkT_sb[:], start=True, stop=True)
                s2_sb = spool.tile([64, 64], F32)
                nc.vector.tensor_mul(s2_sb[:], s_sb[:], rk_bc[:])
                s3 = spool.tile([64, 64], F32)
                nc.vector.tensor_mul(s3[:], s2_sb[:], mask_sb)
                s4 = spool.tile([64, 64], F32)
                nc.vector.tensor_add(s4[:], s3[:], negbig_sb)
                negm = spool.tile([64, 1], F32)
                nc.vector.reduce_max(negm[:], s4[:], axis=AX.X, negate=True)
                ex = spool.tile([64, 64], F32)
                nc.scalar.activation(ex[:], s4[:], AF.Exp, bias=negm[:, 0:1])
                ssum = spool.tile([64, 1], F32)
                nc.vector.reduce_sum(ssum[:], ex[:], axis=AX.X)
                rs = spool.tile([64, 1], F32)
                nc.vector.reciprocal(rs[:], ssum[:])
                ablk = spool.tile([64, 64], F32)
                nc.vector.tensor_scalar_mul(ablk[:], ex[:], rs[:, 0:1])
                ablk2 = spool.tile([64, 64], F32)
                nc.vector.tensor_add(ablk2[:], ablk[:], i128_sb[0:64, 0:64])
                bt_ps = pfin.tile([64, 64], F32, tag="fin")
                nc.tensor.matmul(bt_ps[:], ablk2[:], projT_sb, start=True, stop=True)
                bt_sb = spool.tile([64, 64], BF16)
                nc.scalar.activation(bt_sb[:], bt_ps[:], AF.Copy)

                # ---- final: out = B @ v + proj_b ----
                # 4 outer blocks of 16 rows; batched o_sb, 2 big DMAs each
                for k in range(4):
                    o_sb = opool.tile([128, 8, WL], F32, tag="o_sb")
                    for j in range(4):
                        kk = k * 4 + j
                        o_ps = pfin.tile([128, 512], F32, tag="fin")
                        nc.tensor.matmul(
                            o_ps[0:64, :], bt_sb[:],
                            v_sb[:, kk * 1024 : kk * 1024 + 512],
                            start=True, stop=True,
                        )
                        nc.tensor.matmul(
                            o_ps[64:128, :], bt_sb[:],
                            v_sb[:, kk * 1024 + 512 : kk * 1024 + 1024],
                            start=True, stop=True,
                        )
                        dst = o_sb[:, 2 * j : 2 * j + 2, :]
                        if j % 2 == 0:
                            nc.scalar.activation(
                                dst, o_ps[:], AF.Identity, bias=projb2_sb
                            )
                        else:
                            nc.vector.tensor_scalar_add(dst, o_ps[:], projb2_sb)
                    rows = out_e[:, 16 * k : 16 * k + 16, :].rearrange(
                        "c (j s l) w -> c j s l w", j=4, s=2
                    )
                    nc.sync.dma_start(rows[:, :, 0, :, :], o_sb[0:64, :, :].rearrange(
                        "p (j l) w -> p j l w", j=4
                    ))
                    nc.gpsimd.dma_start(rows[:, :, 1, :, :], o_sb[64:128, :, :].rearrange(
                        "p (j l) w -> p j l w", j=4
                    ))

    nc.finalize()
    return nc


_NC_CACHE = None


def _get_nc():
    global _NC_CACHE
    if _NC_CACHE is None:
        _NC_CACHE = _build_nc()
    return _NC_CACHE


def _host_prep(inputs):
    x = np.asarray(inputs["x"], np.float32)
    dw_w = np.asarray(inputs["dw_w"], np.float32)
    dw_b = np.asarray(inputs["dw_b"], np.float32)
    qkv_w = np.asarray(inputs["qkv_w"], np.float32)
    qkv_b = np.asarray(inputs["qkv_b"], np.float32)
    bn_gamma = np.asarray(inputs["bn_gamma"], np.float32)
    bn_beta = np.asarray(inputs["bn_beta"], np.float32)
    bn_mean = np.asarray(inputs["bn_mean"], np.float32)
    bn_var = np.asarray(inputs["bn_var"], np.float32)
    temperature = np.asarray(inputs["temperature"], np.float32)
    proj_w = np.asarray(inputs["proj_w"], np.float32)
    proj_b = np.asarray(inputs["proj_b"], np.float32)

    # combined 4x4 stride-2 kernels: conv3x3 folded with Haar signs
    K4 = np.zeros((4, 64, 4, 4), np.float32)
    for b_ in range(4):
        for r in range(2):
            for w2 in range(2):
                K4[b_, :, r : r + 3, w2 : w2 + 3] += (
                    SGN[b_, r, w2] * 0.5 * dw_w[:, 0, :, :]
                )

    # lhsT blocks [128=(dlt,c), (bp, tp, tx) * 128=(u, c)]
    bandw = np.zeros((128, 16 * 128), np.float32)
    for bp in range(2):
        for tp in range(2):
            for tx in range(4):
                blk = ((bp * 2 + tp) * 4 + tx) * 128
                for dlt in range(2):
                    ty = 2 * tp + dlt
                    for u in range(2):
                        np.fill_diagonal(
                            bandw[
                                dlt * 64 : (dlt + 1) * 64,
                                blk + u * 64 : blk + (u + 1) * 64,
                            ],
                            K4[bp * 2 + u, :, ty, tx],
                        )

    inv_std = bn_gamma / np.sqrt(bn_var + BN_EPS)
    act_scale = inv_std
    # band bias: yl channels carry +2*dw_b; fold its effect on qkv into the
    # activation bias (qkv sees xcat = bands INCLUDING the ll bias, so the
    # matmul input already contains it -> no change needed there)
    act_bias = qkv_b * inv_std + (bn_beta - bn_mean * inv_std)

    mask = np.zeros((64, 64), np.float32)
    for h in range(HEADS):
        mask[h * 8 : (h + 1) * 8, h * 8 : (h + 1) * 8] = 1.0

    vecs = np.zeros((128, 8), np.float32)
    vecs[:, 0] = act_scale[0:128]
    vecs[:, 1] = act_bias[0:128]
    vecs[0:64, 2] = 2.0 * dw_b
    vecs[0:64, 3] = act_scale[128:192]
    vecs[0:64, 4] = act_bias[128:192]
    vecs[0:64, 5] = np.repeat(temperature.reshape(HEADS), DH)
    vecs[0:64, 6] = proj_b
    vecs[:, 7] = np.tile(proj_b, 2)
    aux_m = np.zeros((128, 256), np.float32)
    aux_m[0:64, 0:64] = proj_w.T
    aux_m[0:64, 64:128] = mask
    aux_m[0:64, 128:192] = (mask - 1.0) * 1e30
    aux_m[0, 192:256] = 1.0

    bf = ml_dtypes.bfloat16
    common = {
        "bandw": bandw.astype(bf),
        "qkvt1": np.ascontiguousarray(qkv_w[:, 0:128].T).astype(bf),
        "qkvt2": np.ascontiguousarray(qkv_w[:, 128:256].T).astype(bf),
        "vecs": vecs,
        "aux": aux_m,
        "i128": np.eye(128, dtype=np.float32),
        "identb": np.eye(128, dtype=np.float32).astype(bf),
    }

    in_maps = []
    for i in range(8):
        b, q = i // 4, i % 4
        xs = np.zeros((C, CH + 2, W), np.float32)
        r0, r1 = q * CH - 1, q * CH + CH + 1
        s0, s1 = max(r0, 0), min(r1, H)
        xs[:, s0 - r0 : s0 - r0 + (s1 - s0), :] = x[b, :, s0:s1, :]
        in_maps.append({"x": xs.astype(bf), **common})
    return in_maps


def run(inputs, trace=False):
    nc = _get_nc()
    in_maps = _host_prep(inputs)
    res = run_bass_kernel_spmd(nc, in_maps, core_ids=list(range(8)), trace=trace)

    out = np.zeros((B, C, HL * 4, WL), np.float32)
    yl = np.zeros((B, C, HL * 4, WL), np.float32)
    yh = np.zeros((B, C, 3, HL * 4, WL), np.float32)
    for i in range(8):
        b, q = i // 4, i % 4
        r = res.results[i]
        sl = slice(q * HL, (q + 1) * HL)
        out[b, :, sl, :] = r["outp"]
        yl[b, :, sl, :] = r["yl"].astype(np.float32)
        yh[b, :, 0, sl, :] = r["lh"].astype(np.float32)
        yh[b, :, 1, sl, :] = r["hl"].astype(np.float32)
        yh[b, :, 2, sl, :] = r["hh"].astype(np.float32)
    return (out, yl, yh), res


def kernel(**inputs):
    (out, yl, yh), _ = run(inputs, trace=False)
    return out, yl, yh
